# revision 4
# baseline (speedup 1.0000x reference)
"""Multi-head attention TRN2 kernel (B=4, S=2048, E=128, H=8) on 8 NeuronCores.

Sharding: core c handles batch b = c // 2 and head group g = c % 2
(heads 4g .. 4g+3).  Each core computes the partial output
outT_partial[e_out, s] = sum_{h in group} (softmax(QK^T/sqrt(E)) V)_h @ Wo_h
for its batch, transposed.  Host sums the two head-group partials per batch,
transposes, and adds bo.

Device algorithm (all-transposed layout, no attention transposes needed):
  qT   [e, s]        via PE transpose of q
  QT_h = Wq_h^T qT   [f, s]  (lhsT = Wq natural layout)
  KT_h likewise      [f, t]
  V_h  = (qT-block)^T Wv_h   [t, f] per 128-block of t
  scoresT[t, s] = KT_h-block^T @ QT  -> exp on ScalarE -> attnT (bf16)
  denom[s] = ones^T @ (DVE-folded attnT)   (column sums)
  ZT[f, s] = sum_t V-block^T... accumulated over t blocks in PSUM
  ZT_norm = ZT * (1/denom);  outT += Wo_h^T @ ZT_norm
"""

import sys

for _p in ("/opt/trn_rl_repo",):
    if _p not in sys.path:
        sys.path.insert(0, _p)

import numpy as np

import concourse.bass as bass
import concourse.mybir as mybir
import concourse.tile as tile
from concourse.bass_utils import run_bass_kernel_spmd
from concourse.masks import make_identity

F32 = mybir.dt.float32
F32R = mybir.dt.float32r
BF16 = mybir.dt.bfloat16

B, S, E, H = 4, 2048, 128, 8
NH = 4          # heads per core
TB = S // 128   # 16 t blocks
S_SPLIT = 2     # s-direction split per head (pipelining unit)
SW = S // S_SPLIT        # 1024
NC_CHUNK = 512           # psum-bank chunk
SCALE = 1.0 / np.sqrt(E)

_prog_cache = {}


def build_program():
    if "nc" in _prog_cache:
        return _prog_cache["nc"]

    import concourse.bacc as bacc

    nc = bacc.Bacc("TRN2", target_bir_lowering=False, debug=False)

    q_d = nc.dram_tensor("q", [S, E], F32, kind="ExternalInput").ap()
    wq_d = nc.dram_tensor("Wq", [NH, E, E], F32, kind="ExternalInput").ap()
    wk_d = nc.dram_tensor("Wk", [NH, E, E], F32, kind="ExternalInput").ap()
    wv_d = nc.dram_tensor("Wv", [NH, E, E], F32, kind="ExternalInput").ap()
    wo_d = nc.dram_tensor("Wo", [NH, E, E], F32, kind="ExternalInput").ap()
    bq_d = nc.dram_tensor("bq", [NH, E], F32, kind="ExternalInput").ap()
    bk_d = nc.dram_tensor("bk", [NH, E], F32, kind="ExternalInput").ap()
    bv_d = nc.dram_tensor("bv", [NH, E], F32, kind="ExternalInput").ap()
    out_d = nc.dram_tensor("out", [E, S], F32, kind="ExternalOutput").ap()

    with tile.TileContext(nc) as tc:
        _emit(nc, tc, q_d, wq_d, wk_d, wv_d, wo_d, bq_d, bk_d, bv_d, out_d)

    nc.compile()
    _prog_cache["nc"] = nc
    return nc


def _emit(nc, tc, q_d, wq_d, wk_d, wv_d, wo_d, bq_d, bk_d, bv_d, out_d):
    from contextlib import ExitStack

    ctx = ExitStack()
    consts = ctx.enter_context(tc.tile_pool(name="consts", bufs=1))
    heads = ctx.enter_context(tc.tile_pool(name="heads", bufs=2))
    attns = ctx.enter_context(tc.tile_pool(name="attns", bufs=2))
    folds = ctx.enter_context(tc.tile_pool(name="folds", bufs=1))
    works = ctx.enter_context(tc.tile_pool(name="works", bufs=2))
    psum_big = ctx.enter_context(tc.tile_pool(name="psum_big", bufs=2, space="PSUM"))
    psum_work = ctx.enter_context(tc.tile_pool(name="psum_work", bufs=2, space="PSUM"))

    # ---- constants / preload ----
    ident = consts.tile([128, 128], F32, tag="ident")
    make_identity(nc, ident)
    ones_bf = consts.tile([128, 128], BF16, tag="ones")
    nc.vector.memset(ones_bf, 1.0)

    w_stage = consts.tile([128, 4, NH, 128], F32, tag="wstage")
    for i, wd in enumerate((wq_d, wk_d, wv_d, wo_d)):
        nc.sync.dma_start(out=w_stage[:, i], in_=wd.rearrange("h i j -> i h j"))
    wq_sb = consts.tile([128, NH, 128], F32R, tag="wq")  # [e_in, h, e_out]
    nc.vector.tensor_copy(wq_sb, w_stage[:, 0])
    wk_sb = consts.tile([128, NH, 128], F32R, tag="wk")
    nc.vector.tensor_copy(wk_sb, w_stage[:, 1])
    wv_sb = consts.tile([128, NH, 128], F32R, tag="wv")
    nc.vector.tensor_copy(wv_sb, w_stage[:, 2])
    wo_sb = consts.tile([128, NH, 128], F32R, tag="wo")  # [f, h, g]
    nc.vector.tensor_copy(wo_sb, w_stage[:, 3])

    bq_sb = consts.tile([128, NH], F32, tag="bq")  # [f, h]
    nc.sync.dma_start(out=bq_sb, in_=bq_d.rearrange("h f -> f h"))
    bk_sb = consts.tile([128, NH], F32, tag="bk")
    nc.sync.dma_start(out=bk_sb, in_=bk_d.rearrange("h f -> f h"))
    # bv broadcast across partitions: [t_sub, h, f]
    bv_b = consts.tile([128, NH, 128], F32, tag="bvb")
    bv_bcast_ap = bass.AP(
        tensor=bv_d.tensor, offset=bv_d.offset, ap=[[0, 128]] + list(bv_d.ap)
    )
    nc.sync.dma_start(out=bv_b, in_=bv_bcast_ap)

    # ---- qT via PE transposes ----
    q_sb = consts.tile([128, TB, 128], F32, tag="qsb")  # [s_sub, s_blk, e]
    nc.sync.dma_start(out=q_sb, in_=q_d.rearrange("(sb p) e -> p sb e", p=128))
    qT = consts.tile([128, TB, 128], F32R, tag="qT")  # [e, s_blk, s_sub]
    for sb in range(TB):
        pt = psum_work.tile([128, 128], F32, tag="work")
        nc.tensor.transpose(pt, q_sb[:, sb, :], ident)
        nc.vector.tensor_copy(qT[:, sb, :], pt)
    qT_flat = qT.rearrange("e sb p -> e (sb p)")  # [e, s]

    # accumulators for output (ping-pong per s-half)
    acc_a = [
        consts.tile([128, SW], F32, tag=f"acc_a{sh}", name=f"acc_a{sh}")
        for sh in range(S_SPLIT)
    ]
    acc_b = [
        consts.tile([128, SW], F32, tag=f"acc_b{sh}", name=f"acc_b{sh}")
        for sh in range(S_SPLIT)
    ]

    for h in range(NH):
        # ---- projections ----
        qt_h = heads.tile([128, S], F32R, tag="QT")  # [f, s]
        kt_h = heads.tile([128, S], F32R, tag="KT")  # [f, t]
        for j in range(S // NC_CHUNK):
            ps = psum_work.tile([128, NC_CHUNK], F32, tag="work")
            nc.tensor.matmul(
                ps,
                lhsT=wq_sb[:, h, :],
                rhs=qT_flat[:, j * NC_CHUNK : (j + 1) * NC_CHUNK],
                start=True,
                stop=True,
            )
            nc.vector.tensor_scalar_add(
                qt_h[:, j * NC_CHUNK : (j + 1) * NC_CHUNK], ps, bq_sb[:, h : h + 1]
            )
        for j in range(S // NC_CHUNK):
            ps = psum_work.tile([128, NC_CHUNK], F32, tag="work")
            nc.tensor.matmul(
                ps,
                lhsT=wk_sb[:, h, :],
                rhs=qT_flat[:, j * NC_CHUNK : (j + 1) * NC_CHUNK],
                start=True,
                stop=True,
            )
            nc.vector.tensor_scalar_add(
                kt_h[:, j * NC_CHUNK : (j + 1) * NC_CHUNK], ps, bk_sb[:, h : h + 1]
            )
        v_h = heads.tile([128, TB, 128], BF16, tag="V")  # [t_sub, t_blk, f]
        for tb in range(TB):
            ps = psum_work.tile([128, 128], F32, tag="work")
            nc.tensor.matmul(
                ps,
                lhsT=qT[:, tb, :],
                rhs=wv_sb[:, h, :],
                start=True,
                stop=True,
            )
            nc.vector.tensor_add(v_h[:, tb, :], ps, bv_b[:, h, :])

        for sh in range(S_SPLIT):
            s0 = sh * SW
            # ---- scores + exp ----
            attnT = attns.tile([128, TB, SW], BF16, tag="attnT")  # [t_sub, t_blk, s]
            for tb in range(TB):
                sc = psum_big.tile([128, SW], F32, tag="sc")
                for j in range(SW // NC_CHUNK):
                    nc.tensor.matmul(
                        sc[:, j * NC_CHUNK : (j + 1) * NC_CHUNK],
                        lhsT=kt_h[:, tb * 128 : (tb + 1) * 128],
                        rhs=qt_h[
                            :, s0 + j * NC_CHUNK : s0 + (j + 1) * NC_CHUNK
                        ],
                        start=True,
                        stop=True,
                    )
                nc.scalar.activation(
                    attnT[:, tb, :], sc, mybir.ActivationFunctionType.Exp, scale=SCALE
                )

            # ---- denominator fold (DVE) ----
            f1 = folds.tile([128, TB // 2, SW], BF16, tag="f1")
            nc.vector.tensor_add(f1, attnT[:, 0 : TB // 2, :], attnT[:, TB // 2 :, :])
            f2 = folds.tile([128, TB // 4, SW], BF16, tag="f2")
            nc.vector.tensor_add(f2, f1[:, 0 : TB // 4, :], f1[:, TB // 4 :, :])

            for c in range(SW // NC_CHUNK):
                c0 = c * NC_CHUNK
                # ---- ZT accumulation over t blocks ----
                zt = psum_work.tile([128, NC_CHUNK], F32, tag="zt", bufs=1)
                for tb in range(TB):
                    nc.tensor.matmul(
                        zt,
                        lhsT=v_h[:, tb, :],
                        rhs=attnT[:, tb, c0 : c0 + NC_CHUNK],
                        start=(tb == 0),
                        stop=(tb == TB - 1),
                    )
                # ---- denominator (column sums of attnT) ----
                dn = psum_work.tile([128, NC_CHUNK], F32, tag="dn", bufs=1)
                for j in range(TB // 4):
                    nc.tensor.matmul(
                        dn,
                        lhsT=ones_bf,
                        rhs=f2[:, j, c0 : c0 + NC_CHUNK],
                        start=(j == 0),
                        stop=(j == TB // 4 - 1),
                    )
                recip = works.tile([128, NC_CHUNK], F32, tag="recip")
                nc.vector.reciprocal_approx_fast(recip, dn)
                ztn = works.tile([128, NC_CHUNK], F32R, tag="ztn")
                nc.vector.tensor_mul(ztn, zt, recip)
                # ---- output projection ----
                wo_ps = psum_work.tile([128, NC_CHUNK], F32, tag="work")
                nc.tensor.matmul(
                    wo_ps,
                    lhsT=wo_sb[:, h, :],
                    rhs=ztn,
                    start=True,
                    stop=True,
                )
                csl = slice(c0, c0 + NC_CHUNK)
                if h == 0:
                    nc.vector.tensor_copy(acc_a[sh][:, csl], wo_ps)
                elif h == 1:
                    nc.vector.tensor_add(acc_b[sh][:, csl], acc_a[sh][:, csl], wo_ps)
                elif h == 2:
                    nc.vector.tensor_add(acc_a[sh][:, csl], acc_b[sh][:, csl], wo_ps)
                else:
                    osb = works.tile([128, NC_CHUNK], F32, tag="osb")
                    nc.vector.tensor_add(osb, acc_a[sh][:, csl], wo_ps)
                    nc.sync.dma_start(out=out_d[:, s0 + c0 : s0 + c0 + NC_CHUNK], in_=osb)

    ctx.close()


def _in_maps(inputs):
    q = np.asarray(inputs["q"], dtype=np.float32)
    Wq = np.asarray(inputs["Wq"], dtype=np.float32)
    bq = np.asarray(inputs["bq"], dtype=np.float32)
    Wk = np.asarray(inputs["Wk"], dtype=np.float32)
    bk = np.asarray(inputs["bk"], dtype=np.float32)
    Wv = np.asarray(inputs["Wv"], dtype=np.float32)
    bv = np.asarray(inputs["bv"], dtype=np.float32)
    Wo = np.asarray(inputs["Wo"], dtype=np.float32).reshape(H, E, E)
    maps = []
    for c in range(8):
        b = c // 2
        hs = slice(4 * (c % 2), 4 * (c % 2) + 4)
        maps.append(
            {
                "q": np.ascontiguousarray(q[b]),
                "Wq": np.ascontiguousarray(Wq[hs]),
                "Wk": np.ascontiguousarray(Wk[hs]),
                "Wv": np.ascontiguousarray(Wv[hs]),
                "Wo": np.ascontiguousarray(Wo[hs]),
                "bq": np.ascontiguousarray(bq[hs]),
                "bk": np.ascontiguousarray(bk[hs]),
                "bv": np.ascontiguousarray(bv[hs]),
            }
        )
    return maps


def kernel(**inputs):
    nc = build_program()
    maps = _in_maps(inputs)
    res = run_bass_kernel_spmd(nc, maps, core_ids=list(range(8)))
    bo = np.asarray(inputs["bo"], dtype=np.float32)
    out = np.empty((B, S, E), dtype=np.float32)
    for b in range(B):
        part = res.results[2 * b]["out"] + res.results[2 * b + 1]["out"]
        out[b] = part.T + bo
    return out


# revision 7
# speedup vs baseline: 1.0832x; 1.0832x over previous
"""Multi-head attention TRN2 kernel (B=4, S=2048, E=128, H=8) on 8 NeuronCores.

Sharding: core c handles batch b = c // 2 and head group g = c % 2
(heads 4g .. 4g+3).  Each core computes the partial output
outT_partial[e_out, s] = sum_{h in group} (softmax(QK^T/sqrt(E)) V)_h @ Wo_h
for its batch, transposed.  Host sums the two head-group partials per batch,
transposes, and adds bo.

Device algorithm (all-transposed layout, no attention transposes needed):
  qT   [e, s]        via PE transpose of q
  QT_h = Wq_h^T qT   [f, s]  (lhsT = Wq natural layout)
  KT_h likewise      [f, t]
  V_h  = (qT-block)^T Wv_h   [t, f] per 128-block of t
  scoresT[t, s] = KT_h-block^T @ QT  -> exp on ScalarE -> attnT (bf16)
  denom[s] = ones^T @ (DVE-folded attnT)   (column sums)
  ZT[f, s] = sum_t V-block^T... accumulated over t blocks in PSUM
  ZT_norm = ZT * (1/denom);  outT += Wo_h^T @ ZT_norm
"""

import sys

for _p in ("/opt/trn_rl_repo",):
    if _p not in sys.path:
        sys.path.insert(0, _p)

import numpy as np

import concourse.bass as bass
import concourse.mybir as mybir
import concourse.tile as tile
from concourse.bass_utils import run_bass_kernel_spmd
from concourse.masks import make_identity

F32 = mybir.dt.float32
F32R = mybir.dt.float32r
BF16 = mybir.dt.bfloat16
F16 = mybir.dt.float16

B, S, E, H = 4, 2048, 128, 8
NH = 4          # heads per core
TB = S // 128   # 16 t blocks
S_SPLIT = 2     # s-direction split per head (pipelining unit)
SW = S // S_SPLIT        # 1024
NC_CHUNK = 512           # psum-bank chunk
SCALE = 1.0 / np.sqrt(E)

_prog_cache = {}


def build_program():
    if "nc" in _prog_cache:
        return _prog_cache["nc"]

    import concourse.bacc as bacc

    nc = bacc.Bacc("TRN2", target_bir_lowering=False, debug=False)

    q_d = nc.dram_tensor("q", [S, E], F32, kind="ExternalInput").ap()
    wq_d = nc.dram_tensor("Wq", [NH, E, E], F32, kind="ExternalInput").ap()
    wk_d = nc.dram_tensor("Wk", [NH, E, E], F32, kind="ExternalInput").ap()
    wv_d = nc.dram_tensor("Wv", [NH, E, E], F32, kind="ExternalInput").ap()
    wo_d = nc.dram_tensor("Wo", [NH, E, E], F32, kind="ExternalInput").ap()
    bq_d = nc.dram_tensor("bq", [NH, E], F32, kind="ExternalInput").ap()
    bk_d = nc.dram_tensor("bk", [NH, E], F32, kind="ExternalInput").ap()
    bv_d = nc.dram_tensor("bv", [NH, E], F32, kind="ExternalInput").ap()
    out_d = nc.dram_tensor("out", [E, S], F32, kind="ExternalOutput").ap()

    with tile.TileContext(nc) as tc:
        _emit(nc, tc, q_d, wq_d, wk_d, wv_d, wo_d, bq_d, bk_d, bv_d, out_d)

    nc.compile()
    _prog_cache["nc"] = nc
    return nc


def _emit(nc, tc, q_d, wq_d, wk_d, wv_d, wo_d, bq_d, bk_d, bv_d, out_d):
    from contextlib import ExitStack

    ctx = ExitStack()
    consts = ctx.enter_context(tc.tile_pool(name="consts", bufs=1))
    heads = ctx.enter_context(tc.tile_pool(name="heads", bufs=2))
    attns = ctx.enter_context(tc.tile_pool(name="attns", bufs=2))
    folds = ctx.enter_context(tc.tile_pool(name="folds", bufs=1))
    works = ctx.enter_context(tc.tile_pool(name="works", bufs=2))
    psum_big = ctx.enter_context(tc.tile_pool(name="psum_big", bufs=2, space="PSUM"))
    psum_work = ctx.enter_context(tc.tile_pool(name="psum_work", bufs=2, space="PSUM"))

    # ---- constants / preload ----
    ident = consts.tile([128, 128], F32, tag="ident")
    make_identity(nc, ident)
    ones_bf = consts.tile([128, 128], F16, tag="ones")
    nc.vector.memset(ones_bf, 1.0)

    # q first: it heads the critical path (transposes -> proj -> scores)
    q_sb = consts.tile([128, TB, 128], F32, tag="qsb")  # [s_sub, s_blk, e]
    nc.sync.dma_start(out=q_sb, in_=q_d.rearrange("(sb p) e -> p sb e", p=128))

    w_stage = consts.tile([128, 4, NH, 128], F32, tag="wstage")
    for i, wd in enumerate((wq_d, wk_d, wv_d, wo_d)):
        nc.sync.dma_start(out=w_stage[:, i], in_=wd.rearrange("h i j -> i h j"))
    wq_sb = consts.tile([128, NH, 128], F16, tag="wq")  # [e_in, h, e_out]
    nc.vector.tensor_copy(wq_sb, w_stage[:, 0])
    wk_sb = consts.tile([128, NH, 128], F16, tag="wk")
    nc.vector.tensor_copy(wk_sb, w_stage[:, 1])
    wv_sb = consts.tile([128, NH, 128], F16, tag="wv")
    nc.vector.tensor_copy(wv_sb, w_stage[:, 2])
    wo_sb = consts.tile([128, NH, 128], F16, tag="wo")  # [f, h, g]
    nc.vector.tensor_copy(wo_sb, w_stage[:, 3])

    bq_sb = consts.tile([128, NH], F32, tag="bq")  # [f, h]
    nc.sync.dma_start(out=bq_sb, in_=bq_d.rearrange("h f -> f h"))
    bk_sb = consts.tile([128, NH], F32, tag="bk")
    nc.sync.dma_start(out=bk_sb, in_=bk_d.rearrange("h f -> f h"))
    # bv broadcast across partitions: [t_sub, h, f]
    bv_b = consts.tile([128, NH, 128], F32, tag="bvb")
    bv_bcast_ap = bass.AP(
        tensor=bv_d.tensor, offset=bv_d.offset, ap=[[0, 128]] + list(bv_d.ap)
    )
    nc.sync.dma_start(out=bv_b, in_=bv_bcast_ap)

    # ---- qT via PE transposes ----
    qT = consts.tile([128, TB, 128], F16, tag="qT")  # [e, s_blk, s_sub]
    for sb in range(TB):
        pt = psum_work.tile([128, 128], F32, tag="work")
        nc.tensor.transpose(pt, q_sb[:, sb, :], ident)
        nc.vector.tensor_copy(qT[:, sb, :], pt)
    qT_flat = qT.rearrange("e sb p -> e (sb p)")  # [e, s]

    # accumulators for output (ping-pong per s-half)
    acc_a = [
        consts.tile([128, SW], F32, tag=f"acc_a{sh}", name=f"acc_a{sh}")
        for sh in range(S_SPLIT)
    ]
    acc_b = [
        consts.tile([128, SW], F32, tag=f"acc_b{sh}", name=f"acc_b{sh}")
        for sh in range(S_SPLIT)
    ]

    for h in range(NH):
        # ---- projections ----
        qt_h = heads.tile([128, S], F16, tag="QT")  # [f, s]
        kt_h = heads.tile([128, S], F16, tag="KT")  # [f, t]
        for j in range(S // NC_CHUNK):
            ps = psum_work.tile([128, NC_CHUNK], F32, tag="work")
            nc.tensor.matmul(
                ps,
                lhsT=wq_sb[:, h, :],
                rhs=qT_flat[:, j * NC_CHUNK : (j + 1) * NC_CHUNK],
                start=True,
                stop=True,
            )
            nc.vector.tensor_scalar_add(
                qt_h[:, j * NC_CHUNK : (j + 1) * NC_CHUNK], ps, bq_sb[:, h : h + 1]
            )
        for j in range(S // NC_CHUNK):
            ps = psum_work.tile([128, NC_CHUNK], F32, tag="work")
            nc.tensor.matmul(
                ps,
                lhsT=wk_sb[:, h, :],
                rhs=qT_flat[:, j * NC_CHUNK : (j + 1) * NC_CHUNK],
                start=True,
                stop=True,
            )
            nc.vector.tensor_scalar_add(
                kt_h[:, j * NC_CHUNK : (j + 1) * NC_CHUNK], ps, bk_sb[:, h : h + 1]
            )
        v_h = heads.tile([128, TB, 128], F16, tag="V")  # [t_sub, t_blk, f]
        for tb in range(TB):
            ps = psum_work.tile([128, 128], F32, tag="work")
            nc.tensor.matmul(
                ps,
                lhsT=qT[:, tb, :],
                rhs=wv_sb[:, h, :],
                start=True,
                stop=True,
            )
            nc.vector.tensor_add(v_h[:, tb, :], ps, bv_b[:, h, :])

        for sh in range(S_SPLIT):
            s0 = sh * SW
            # ---- scores + exp ----
            attnT = attns.tile([128, TB, SW], F16, tag="attnT")  # [t_sub, t_blk, s]
            for tb in range(TB):
                sc = psum_big.tile([128, SW], F32, tag="sc")
                for j in range(SW // NC_CHUNK):
                    nc.tensor.matmul(
                        sc[:, j * NC_CHUNK : (j + 1) * NC_CHUNK],
                        lhsT=kt_h[:, tb * 128 : (tb + 1) * 128],
                        rhs=qt_h[
                            :, s0 + j * NC_CHUNK : s0 + (j + 1) * NC_CHUNK
                        ],
                        start=True,
                        stop=True,
                    )
                nc.scalar.activation(
                    attnT[:, tb, :], sc, mybir.ActivationFunctionType.Exp, scale=SCALE
                )

            # ---- denominator fold (DVE) ----
            f1 = folds.tile([128, TB // 2, SW], F16, tag="f1")
            nc.vector.tensor_add(f1, attnT[:, 0 : TB // 2, :], attnT[:, TB // 2 :, :])
            f2 = folds.tile([128, TB // 4, SW], F16, tag="f2")
            nc.vector.tensor_add(f2, f1[:, 0 : TB // 4, :], f1[:, TB // 4 :, :])
            f3 = folds.tile([128, TB // 8, SW], F16, tag="f3")
            nc.vector.tensor_add(f3, f2[:, 0 : TB // 8, :], f2[:, TB // 8 :, :])
            f4 = folds.tile([128, 1, SW], F16, tag="f4")
            nc.vector.tensor_add(f4, f3[:, 0:1, :], f3[:, 1:2, :])

            for c in range(SW // NC_CHUNK):
                c0 = c * NC_CHUNK
                # ---- ZT accumulation over t blocks ----
                zt = psum_work.tile([128, NC_CHUNK], F32, tag="zt", bufs=1)
                for tb in range(TB):
                    nc.tensor.matmul(
                        zt,
                        lhsT=v_h[:, tb, :],
                        rhs=attnT[:, tb, c0 : c0 + NC_CHUNK],
                        start=(tb == 0),
                        stop=(tb == TB - 1),
                    )
                # ---- denominator (column sums of attnT) ----
                dn = psum_work.tile([128, NC_CHUNK], F32, tag="work")
                nc.tensor.matmul(
                    dn,
                    lhsT=ones_bf,
                    rhs=f4[:, 0, c0 : c0 + NC_CHUNK],
                    start=True,
                    stop=True,
                )
                recip = works.tile([128, NC_CHUNK], F32, tag="recip")
                nc.vector.reciprocal_approx_fast(recip, dn)
                ztn = works.tile([128, NC_CHUNK], F16, tag="ztn")
                nc.vector.tensor_mul(ztn, zt, recip)
                # ---- output projection ----
                wo_ps = psum_work.tile([128, NC_CHUNK], F32, tag="wops", bufs=1)
                nc.tensor.matmul(
                    wo_ps,
                    lhsT=wo_sb[:, h, :],
                    rhs=ztn,
                    start=True,
                    stop=True,
                )
                csl = slice(c0, c0 + NC_CHUNK)
                if h == 0:
                    nc.vector.tensor_copy(acc_a[sh][:, csl], wo_ps)
                elif h == 1:
                    nc.vector.tensor_add(acc_b[sh][:, csl], acc_a[sh][:, csl], wo_ps)
                elif h == 2:
                    nc.vector.tensor_add(acc_a[sh][:, csl], acc_b[sh][:, csl], wo_ps)
                else:
                    osb = works.tile([128, NC_CHUNK], F32, tag="osb")
                    nc.vector.tensor_add(osb, acc_a[sh][:, csl], wo_ps)
                    nc.sync.dma_start(out=out_d[:, s0 + c0 : s0 + c0 + NC_CHUNK], in_=osb)

    ctx.close()


def _in_maps(inputs):
    q = np.asarray(inputs["q"], dtype=np.float32)
    Wq = np.asarray(inputs["Wq"], dtype=np.float32)
    bq = np.asarray(inputs["bq"], dtype=np.float32)
    Wk = np.asarray(inputs["Wk"], dtype=np.float32)
    bk = np.asarray(inputs["bk"], dtype=np.float32)
    Wv = np.asarray(inputs["Wv"], dtype=np.float32)
    bv = np.asarray(inputs["bv"], dtype=np.float32)
    Wo = np.asarray(inputs["Wo"], dtype=np.float32).reshape(H, E, E)
    maps = []
    for c in range(8):
        b = c // 2
        hs = slice(4 * (c % 2), 4 * (c % 2) + 4)
        maps.append(
            {
                "q": np.ascontiguousarray(q[b]),
                "Wq": np.ascontiguousarray(Wq[hs]),
                "Wk": np.ascontiguousarray(Wk[hs]),
                "Wv": np.ascontiguousarray(Wv[hs]),
                "Wo": np.ascontiguousarray(Wo[hs]),
                "bq": np.ascontiguousarray(bq[hs]),
                "bk": np.ascontiguousarray(bk[hs]),
                "bv": np.ascontiguousarray(bv[hs]),
            }
        )
    return maps


def kernel(**inputs):
    nc = build_program()
    maps = _in_maps(inputs)
    res = run_bass_kernel_spmd(nc, maps, core_ids=list(range(8)))
    bo = np.asarray(inputs["bo"], dtype=np.float32)
    out = np.empty((B, S, E), dtype=np.float32)
    for b in range(B):
        part = res.results[2 * b]["out"] + res.results[2 * b + 1]["out"]
        out[b] = part.T + bo
    return out


# revision 8
# speedup vs baseline: 1.2851x; 1.1864x over previous
"""Multi-head attention TRN2 kernel (B=4, S=2048, E=128, H=8) on 8 NeuronCores.

Sharding: core c handles batch b = c // 2 and head group g = c % 2
(heads 4g .. 4g+3).  Each core computes the partial output
outT_partial[e_out, s] = sum_{h in group} (softmax(QK^T/sqrt(E)) V)_h @ Wo_h
for its batch, transposed.  Host sums the two head-group partials per batch,
transposes, and adds bo.

Device algorithm (all-transposed layout, no attention transposes needed):
  qT   [e, s]        via PE transpose of q
  QT_h = Wq_h^T qT   [f, s]  (lhsT = Wq natural layout)
  KT_h likewise      [f, t]
  V_h  = (qT-block)^T Wv_h   [t, f] per 128-block of t
  scoresT[t, s] = KT_h-block^T @ QT  -> exp on ScalarE -> attnT (bf16)
  denom[s] = ones^T @ (DVE-folded attnT)   (column sums)
  ZT[f, s] = sum_t V-block^T... accumulated over t blocks in PSUM
  ZT_norm = ZT * (1/denom);  outT += Wo_h^T @ ZT_norm
"""

import sys

for _p in ("/opt/trn_rl_repo",):
    if _p not in sys.path:
        sys.path.insert(0, _p)

import numpy as np

import concourse.bass as bass
import concourse.mybir as mybir
import concourse.tile as tile
from concourse.bass_utils import run_bass_kernel_spmd
from concourse.masks import make_identity

F32 = mybir.dt.float32
F32R = mybir.dt.float32r
BF16 = mybir.dt.bfloat16
F16 = mybir.dt.float16

B, S, E, H = 4, 2048, 128, 8
NH = 4          # heads per core
TB = S // 128   # 16 t blocks
S_SPLIT = 2     # s-direction split per head (pipelining unit)
SW = S // S_SPLIT        # 1024
NC_CHUNK = 512           # psum-bank chunk
SCALE = 1.0 / np.sqrt(E)

_prog_cache = {}


def build_program():
    if "nc" in _prog_cache:
        return _prog_cache["nc"]

    import concourse.bacc as bacc

    nc = bacc.Bacc("TRN2", target_bir_lowering=False, debug=False)

    q_d = nc.dram_tensor("q", [S, E], F32, kind="ExternalInput").ap()
    wq_d = nc.dram_tensor("Wq", [NH, E, E], F32, kind="ExternalInput").ap()
    wk_d = nc.dram_tensor("Wk", [NH, E, E], F32, kind="ExternalInput").ap()
    wv_d = nc.dram_tensor("Wv", [NH, E, E], F32, kind="ExternalInput").ap()
    wo_d = nc.dram_tensor("Wo", [NH, E, E], F32, kind="ExternalInput").ap()
    bq_d = nc.dram_tensor("bq", [NH, E], F32, kind="ExternalInput").ap()
    bk_d = nc.dram_tensor("bk", [NH, E], F32, kind="ExternalInput").ap()
    bv_d = nc.dram_tensor("bv", [NH, E], F32, kind="ExternalInput").ap()
    out_d = nc.dram_tensor("out", [E, S], F32, kind="ExternalOutput").ap()

    with tile.TileContext(nc) as tc:
        _emit(nc, tc, q_d, wq_d, wk_d, wv_d, wo_d, bq_d, bk_d, bv_d, out_d)

    nc.compile()
    _prog_cache["nc"] = nc
    return nc


def _emit(nc, tc, q_d, wq_d, wk_d, wv_d, wo_d, bq_d, bk_d, bv_d, out_d):
    from contextlib import ExitStack

    ctx = ExitStack()
    consts = ctx.enter_context(tc.tile_pool(name="consts", bufs=1))
    heads = ctx.enter_context(tc.tile_pool(name="heads", bufs=2))
    attns = ctx.enter_context(tc.tile_pool(name="attns", bufs=2))
    folds = ctx.enter_context(tc.tile_pool(name="folds", bufs=1))
    works = ctx.enter_context(tc.tile_pool(name="works", bufs=2))
    psum_big = ctx.enter_context(tc.tile_pool(name="psum_big", bufs=2, space="PSUM"))
    psum_work = ctx.enter_context(tc.tile_pool(name="psum_work", bufs=2, space="PSUM"))

    # ---- constants / preload ----
    ident = consts.tile([128, 128], F32, tag="ident")
    make_identity(nc, ident)
    ones_bf = consts.tile([128, 128], F16, tag="ones")
    nc.vector.memset(ones_bf, 1.0)

    # q first: it heads the critical path (transposes -> proj -> scores)
    q_sb = consts.tile([128, TB, 128], F32, tag="qsb")  # [s_sub, s_blk, e]
    q_r = q_d.rearrange("(sb p) e -> p sb e", p=128)
    for qc in range(4):
        nc.sync.dma_start(
            out=q_sb[:, qc * (TB // 4) : (qc + 1) * (TB // 4), :],
            in_=q_r[:, qc * (TB // 4) : (qc + 1) * (TB // 4), :],
        )

    w_stage = consts.tile([128, 4, NH, 128], F32, tag="wstage")
    for i, wd in enumerate((wq_d, wk_d, wv_d, wo_d)):
        nc.sync.dma_start(out=w_stage[:, i], in_=wd.rearrange("h i j -> i h j"))
    wq_sb = consts.tile([128, NH, 128], F16, tag="wq")  # [e_in, h, e_out]
    nc.vector.tensor_copy(wq_sb, w_stage[:, 0])
    wk_sb = consts.tile([128, NH, 128], F16, tag="wk")
    nc.vector.tensor_copy(wk_sb, w_stage[:, 1])
    wv_sb = consts.tile([128, NH, 128], F16, tag="wv")
    nc.vector.tensor_copy(wv_sb, w_stage[:, 2])
    wo_sb = consts.tile([128, NH, 128], F16, tag="wo")  # [f, h, g]
    nc.vector.tensor_copy(wo_sb, w_stage[:, 3])

    bq_sb = consts.tile([128, NH], F32, tag="bq")  # [f, h]
    nc.sync.dma_start(out=bq_sb, in_=bq_d.rearrange("h f -> f h"))
    bk_sb = consts.tile([128, NH], F32, tag="bk")
    nc.sync.dma_start(out=bk_sb, in_=bk_d.rearrange("h f -> f h"))
    # bv broadcast across partitions: [t_sub, h, f]
    bv_b = consts.tile([128, NH, 128], F32, tag="bvb")
    bv_bcast_ap = bass.AP(
        tensor=bv_d.tensor, offset=bv_d.offset, ap=[[0, 128]] + list(bv_d.ap)
    )
    nc.sync.dma_start(out=bv_b, in_=bv_bcast_ap)

    # ---- qT via PE transposes ----
    qT = consts.tile([128, TB, 128], F16, tag="qT")  # [e, s_blk, s_sub]
    for sb in range(TB):
        pt = psum_work.tile([128, 128], F32, tag="work")
        nc.tensor.transpose(pt, q_sb[:, sb, :], ident)
        nc.vector.tensor_copy(qT[:, sb, :], pt)
    qT_flat = qT.rearrange("e sb p -> e (sb p)")  # [e, s]

    # accumulators for output (ping-pong per s-half)
    acc_a = [
        consts.tile([128, SW], F32, tag=f"acc_a{sh}", name=f"acc_a{sh}")
        for sh in range(S_SPLIT)
    ]
    acc_b = [
        consts.tile([128, SW], F32, tag=f"acc_b{sh}", name=f"acc_b{sh}")
        for sh in range(S_SPLIT)
    ]

    for h in range(NH):
        # ---- projections ----
        qt_h = heads.tile([128, S], F16, tag="QT")  # [f, s]
        kt_h = heads.tile([128, S], F16, tag="KT")  # [f, t]
        for j in range(S // NC_CHUNK):
            ps = psum_work.tile([128, NC_CHUNK], F32, tag="work")
            nc.tensor.matmul(
                ps,
                lhsT=wq_sb[:, h, :],
                rhs=qT_flat[:, j * NC_CHUNK : (j + 1) * NC_CHUNK],
                start=True,
                stop=True,
            )
            nc.vector.tensor_scalar_add(
                qt_h[:, j * NC_CHUNK : (j + 1) * NC_CHUNK], ps, bq_sb[:, h : h + 1]
            )
        for j in range(S // NC_CHUNK):
            ps = psum_work.tile([128, NC_CHUNK], F32, tag="work")
            nc.tensor.matmul(
                ps,
                lhsT=wk_sb[:, h, :],
                rhs=qT_flat[:, j * NC_CHUNK : (j + 1) * NC_CHUNK],
                start=True,
                stop=True,
            )
            nc.vector.tensor_scalar_add(
                kt_h[:, j * NC_CHUNK : (j + 1) * NC_CHUNK], ps, bk_sb[:, h : h + 1]
            )
        v_h = heads.tile([128, TB, 128], F16, tag="V")  # [t_sub, t_blk, f]
        for tb in range(TB):
            ps = psum_work.tile([128, 128], F32, tag="work")
            nc.tensor.matmul(
                ps,
                lhsT=qT[:, tb, :],
                rhs=wv_sb[:, h, :],
                start=True,
                stop=True,
            )
            nc.vector.tensor_add(v_h[:, tb, :], ps, bv_b[:, h, :])

        for sh in range(S_SPLIT):
            s0 = sh * SW
            # ---- scores + exp ----
            attnT = attns.tile([128, TB, SW], F16, tag="attnT")  # [t_sub, t_blk, s]
            # denominator fold tiles; pairwise adds interleave with the exp
            # stream so only a short chain trails the last exp
            f1 = folds.tile([128, TB // 2, SW], F16, tag="f1")
            f2 = folds.tile([128, TB // 4, SW], F16, tag="f2")
            f3 = folds.tile([128, TB // 8, SW], F16, tag="f3")
            f4 = folds.tile([128, 1, SW], F16, tag="f4")
            for tb in range(TB):
                sc = psum_big.tile([128, SW], F32, tag="sc")
                for j in range(SW // NC_CHUNK):
                    nc.tensor.matmul(
                        sc[:, j * NC_CHUNK : (j + 1) * NC_CHUNK],
                        lhsT=kt_h[:, tb * 128 : (tb + 1) * 128],
                        rhs=qt_h[
                            :, s0 + j * NC_CHUNK : s0 + (j + 1) * NC_CHUNK
                        ],
                        start=True,
                        stop=True,
                    )
                nc.scalar.activation(
                    attnT[:, tb, :], sc, mybir.ActivationFunctionType.Exp, scale=SCALE
                )
                if tb >= 8:
                    i = tb - 8
                    nc.vector.tensor_add(
                        f1[:, i, :], attnT[:, i, :], attnT[:, tb, :]
                    )
                if tb >= 12:
                    i = tb - 12
                    nc.vector.tensor_add(f2[:, i, :], f1[:, i, :], f1[:, i + 4, :])
                if tb >= 14:
                    i = tb - 14
                    nc.vector.tensor_add(f3[:, i, :], f2[:, i, :], f2[:, i + 2, :])
                if tb == 15:
                    nc.vector.tensor_add(f4[:, 0, :], f3[:, 0, :], f3[:, 1, :])

            for c in range(SW // NC_CHUNK):
                c0 = c * NC_CHUNK
                # ---- ZT accumulation over t blocks ----
                zt = psum_work.tile([128, NC_CHUNK], F32, tag="zt")
                for tb in range(TB):
                    nc.tensor.matmul(
                        zt,
                        lhsT=v_h[:, tb, :],
                        rhs=attnT[:, tb, c0 : c0 + NC_CHUNK],
                        start=(tb == 0),
                        stop=(tb == TB - 1),
                    )
                # ---- denominator (column sums of attnT) ----
                dn = psum_work.tile([128, NC_CHUNK], F32, tag="work")
                nc.tensor.matmul(
                    dn,
                    lhsT=ones_bf,
                    rhs=f4[:, 0, c0 : c0 + NC_CHUNK],
                    start=True,
                    stop=True,
                )
                recip = works.tile([128, NC_CHUNK], F32, tag="recip")
                nc.vector.reciprocal_approx_fast(recip, dn)
                ztn = works.tile([128, NC_CHUNK], F16, tag="ztn")
                nc.vector.tensor_mul(ztn, zt, recip)
                # ---- output projection ----
                wo_ps = psum_work.tile([128, NC_CHUNK], F32, tag="work")
                nc.tensor.matmul(
                    wo_ps,
                    lhsT=wo_sb[:, h, :],
                    rhs=ztn,
                    start=True,
                    stop=True,
                )
                csl = slice(c0, c0 + NC_CHUNK)
                if h == 0:
                    nc.vector.tensor_copy(acc_a[sh][:, csl], wo_ps)
                elif h == 1:
                    nc.vector.tensor_add(acc_b[sh][:, csl], acc_a[sh][:, csl], wo_ps)
                elif h == 2:
                    nc.vector.tensor_add(acc_a[sh][:, csl], acc_b[sh][:, csl], wo_ps)
                else:
                    osb = works.tile([128, NC_CHUNK], F32, tag="osb")
                    nc.vector.tensor_add(osb, acc_a[sh][:, csl], wo_ps)
                    nc.sync.dma_start(out=out_d[:, s0 + c0 : s0 + c0 + NC_CHUNK], in_=osb)

    ctx.close()


def _in_maps(inputs):
    q = np.asarray(inputs["q"], dtype=np.float32)
    Wq = np.asarray(inputs["Wq"], dtype=np.float32)
    bq = np.asarray(inputs["bq"], dtype=np.float32)
    Wk = np.asarray(inputs["Wk"], dtype=np.float32)
    bk = np.asarray(inputs["bk"], dtype=np.float32)
    Wv = np.asarray(inputs["Wv"], dtype=np.float32)
    bv = np.asarray(inputs["bv"], dtype=np.float32)
    Wo = np.asarray(inputs["Wo"], dtype=np.float32).reshape(H, E, E)
    maps = []
    for c in range(8):
        b = c // 2
        hs = slice(4 * (c % 2), 4 * (c % 2) + 4)
        maps.append(
            {
                "q": np.ascontiguousarray(q[b]),
                "Wq": np.ascontiguousarray(Wq[hs]),
                "Wk": np.ascontiguousarray(Wk[hs]),
                "Wv": np.ascontiguousarray(Wv[hs]),
                "Wo": np.ascontiguousarray(Wo[hs]),
                "bq": np.ascontiguousarray(bq[hs]),
                "bk": np.ascontiguousarray(bk[hs]),
                "bv": np.ascontiguousarray(bv[hs]),
            }
        )
    return maps


def kernel(**inputs):
    nc = build_program()
    maps = _in_maps(inputs)
    res = run_bass_kernel_spmd(nc, maps, core_ids=list(range(8)))
    bo = np.asarray(inputs["bo"], dtype=np.float32)
    out = np.empty((B, S, E), dtype=np.float32)
    for b in range(B):
        part = res.results[2 * b]["out"] + res.results[2 * b + 1]["out"]
        out[b] = part.T + bo
    return out


# revision 9
# speedup vs baseline: 1.4222x; 1.1067x over previous
"""Multi-head attention TRN2 kernel (B=4, S=2048, E=128, H=8) on 8 NeuronCores.

Sharding: core c handles batch b = c // 2 and head group g = c % 2
(heads 4g .. 4g+3).  Each core computes the partial output
outT_partial[e_out, s] = sum_{h in group} (softmax(QK^T/sqrt(E)) V)_h @ Wo_h
for its batch, transposed.  Host sums the two head-group partials per batch,
transposes, and adds bo.

Device algorithm (all-transposed layout, no attention transposes needed):
  qT   [e, s]        via PE transpose of q
  QT_h = Wq_h^T qT   [f, s]  (lhsT = Wq natural layout)
  KT_h likewise      [f, t]
  V_h  = (qT-block)^T Wv_h   [t, f] per 128-block of t
  scoresT[t, s] = KT_h-block^T @ QT  -> exp on ScalarE -> attnT (bf16)
  denom[s] = ones^T @ (DVE-folded attnT)   (column sums)
  ZT[f, s] = sum_t V-block^T... accumulated over t blocks in PSUM
  ZT_norm = ZT * (1/denom);  outT += Wo_h^T @ ZT_norm
"""

import sys

for _p in ("/opt/trn_rl_repo",):
    if _p not in sys.path:
        sys.path.insert(0, _p)

import numpy as np

import concourse.bass as bass
import concourse.mybir as mybir
import concourse.tile as tile
from concourse.bass_utils import run_bass_kernel_spmd
from concourse.masks import make_identity

F32 = mybir.dt.float32
F32R = mybir.dt.float32r
BF16 = mybir.dt.bfloat16
F16 = mybir.dt.float16

B, S, E, H = 4, 2048, 128, 8
NH = 4          # heads per core
TB = S // 128   # 16 t blocks
S_SPLIT = 2     # s-direction split per head (pipelining unit)
SW = S // S_SPLIT        # 1024
NC_CHUNK = 512           # psum-bank chunk
SCALE = 1.0 / np.sqrt(E)

_prog_cache = {}


def build_program():
    if "nc" in _prog_cache:
        return _prog_cache["nc"]

    import concourse.bacc as bacc

    nc = bacc.Bacc("TRN2", target_bir_lowering=False, debug=False)

    q_d = nc.dram_tensor("q", [S, E], F32, kind="ExternalInput").ap()
    wq_d = nc.dram_tensor("Wq", [NH, E, E], F32, kind="ExternalInput").ap()
    wk_d = nc.dram_tensor("Wk", [NH, E, E], F32, kind="ExternalInput").ap()
    wv_d = nc.dram_tensor("Wv", [NH, E, E], F32, kind="ExternalInput").ap()
    wo_d = nc.dram_tensor("Wo", [NH, E, E], F32, kind="ExternalInput").ap()
    bq_d = nc.dram_tensor("bq", [NH, E], F32, kind="ExternalInput").ap()
    bk_d = nc.dram_tensor("bk", [NH, E], F32, kind="ExternalInput").ap()
    out_d = nc.dram_tensor("out", [E, S], F32, kind="ExternalOutput").ap()

    with tile.TileContext(nc) as tc:
        _emit(nc, tc, q_d, wq_d, wk_d, wv_d, wo_d, bq_d, bk_d, out_d)

    nc.compile()
    _prog_cache["nc"] = nc
    return nc


def _emit(nc, tc, q_d, wq_d, wk_d, wv_d, wo_d, bq_d, bk_d, out_d):
    from contextlib import ExitStack

    ctx = ExitStack()
    consts = ctx.enter_context(tc.tile_pool(name="consts", bufs=1))
    heads = ctx.enter_context(tc.tile_pool(name="heads", bufs=2))
    attns = ctx.enter_context(tc.tile_pool(name="attns", bufs=2))
    folds = ctx.enter_context(tc.tile_pool(name="folds", bufs=1))
    works = ctx.enter_context(tc.tile_pool(name="works", bufs=2))
    psum_big = ctx.enter_context(tc.tile_pool(name="psum_big", bufs=2, space="PSUM"))
    psum_work = ctx.enter_context(tc.tile_pool(name="psum_work", bufs=2, space="PSUM"))

    # ---- constants / preload ----
    ident = consts.tile([128, 128], F32, tag="ident")
    make_identity(nc, ident)
    ones_bf = consts.tile([128, 128], F16, tag="ones")
    nc.vector.memset(ones_bf, 1.0)

    # q first: it heads the critical path (transposes -> proj -> scores)
    q_sb = consts.tile([128, TB, 128], F32, tag="qsb")  # [s_sub, s_blk, e]
    q_r = q_d.rearrange("(sb p) e -> p sb e", p=128)
    for qc in range(4):
        nc.sync.dma_start(
            out=q_sb[:, qc * (TB // 4) : (qc + 1) * (TB // 4), :],
            in_=q_r[:, qc * (TB // 4) : (qc + 1) * (TB // 4), :],
        )

    w_stage = consts.tile([128, 4, NH, 128], F32, tag="wstage")
    for i, wd in enumerate((wq_d, wk_d, wv_d, wo_d)):
        nc.sync.dma_start(out=w_stage[:, i], in_=wd.rearrange("h i j -> i h j"))
    wq_sb = consts.tile([128, NH, 128], F16, tag="wq")  # [e_in, h, e_out]
    nc.vector.tensor_copy(wq_sb, w_stage[:, 0])
    wk_sb = consts.tile([128, NH, 128], F16, tag="wk")
    nc.vector.tensor_copy(wk_sb, w_stage[:, 1])
    wv_sb = consts.tile([128, NH, 128], F16, tag="wv")
    nc.vector.tensor_copy(wv_sb, w_stage[:, 2])
    wo_sb = consts.tile([128, NH, 128], F16, tag="wo")  # [f, h, g]
    nc.vector.tensor_copy(wo_sb, w_stage[:, 3])

    bq_sb = consts.tile([128, NH], F32, tag="bq")  # [f, h]
    nc.sync.dma_start(out=bq_sb, in_=bq_d.rearrange("h f -> f h"))
    bk_sb = consts.tile([128, NH], F32, tag="bk")
    nc.sync.dma_start(out=bk_sb, in_=bk_d.rearrange("h f -> f h"))

    # ---- qT via PE transposes ----
    qT = consts.tile([128, TB, 128], F16, tag="qT")  # [e, s_blk, s_sub]
    for sb in range(TB):
        pt = psum_work.tile([128, 128], F32, tag="work")
        nc.tensor.transpose(pt, q_sb[:, sb, :], ident)
        nc.vector.tensor_copy(qT[:, sb, :], pt)
    qT_flat = qT.rearrange("e sb p -> e (sb p)")  # [e, s]

    # accumulators for output (ping-pong per s-half)
    acc_a = [
        consts.tile([128, SW], F32, tag=f"acc_a{sh}", name=f"acc_a{sh}")
        for sh in range(S_SPLIT)
    ]
    acc_b = [
        consts.tile([128, SW], F32, tag=f"acc_b{sh}", name=f"acc_b{sh}")
        for sh in range(S_SPLIT)
    ]

    def emit_proj(h):
        # ---- projections for head h ----
        qt_h = heads.tile([128, S], F16, tag="QT", name=f"qt_{h}")  # [f, s]
        kt_h = heads.tile([128, S], F16, tag="KT", name=f"kt_{h}")  # [f, t]
        for j in range(S // NC_CHUNK):
            ps = psum_work.tile([128, NC_CHUNK], F32, tag="work", name=f"qtp_{h}_{j}")
            nc.tensor.matmul(
                ps,
                lhsT=wq_sb[:, h, :],
                rhs=qT_flat[:, j * NC_CHUNK : (j + 1) * NC_CHUNK],
                start=True,
                stop=True,
            )
            nc.vector.tensor_scalar_add(
                qt_h[:, j * NC_CHUNK : (j + 1) * NC_CHUNK], ps, bq_sb[:, h : h + 1]
            )
        for j in range(S // NC_CHUNK):
            ps = psum_work.tile([128, NC_CHUNK], F32, tag="work", name=f"ktp_{h}_{j}")
            nc.tensor.matmul(
                ps,
                lhsT=wk_sb[:, h, :],
                rhs=qT_flat[:, j * NC_CHUNK : (j + 1) * NC_CHUNK],
                start=True,
                stop=True,
            )
            nc.vector.tensor_scalar_add(
                kt_h[:, j * NC_CHUNK : (j + 1) * NC_CHUNK], ps, bk_sb[:, h : h + 1]
            )
        v_h = heads.tile([128, TB, 128], F16, tag="V", name=f"v_{h}")  # [t_sub, t_blk, f]
        for g4 in range(TB // 4):
            vps = psum_work.tile([128, NC_CHUNK], F32, tag="work", name=f"vp_{h}_{g4}")
            for k in range(4):
                tb = g4 * 4 + k
                nc.tensor.matmul(
                    vps[:, k * 128 : (k + 1) * 128],
                    lhsT=qT[:, tb, :],
                    rhs=wv_sb[:, h, :],
                    start=True,
                    stop=True,
                )
            nc.vector.tensor_copy(
                v_h.rearrange("p t f -> p (t f)")[:, g4 * 512 : (g4 + 1) * 512], vps
            )
        return qt_h, kt_h, v_h

    proj = {0: emit_proj(0)}
    for h in range(NH):
        qt_h, kt_h, v_h = proj.pop(h)
        for sh in range(S_SPLIT):
            if sh == 1 and h + 1 < NH:
                proj[h + 1] = emit_proj(h + 1)
            s0 = sh * SW
            # ---- scores + exp + fold + AV, interleaved per t-block ----
            attnT = attns.tile([128, TB, SW], F16, tag="attnT")  # [t_sub, t_blk, s]
            f1 = folds.tile([128, TB // 2, SW], F16, tag="f1")
            f2 = folds.tile([128, TB // 4, SW], F16, tag="f2")
            zts = [
                psum_work.tile([128, NC_CHUNK], F32, tag="zt", name=f"zt_{h}_{sh}_{c}")
                for c in range(SW // NC_CHUNK)
            ]
            for tb in range(TB):
                sc = psum_big.tile([128, SW], F32, tag="sc")
                for j in range(SW // NC_CHUNK):
                    nc.tensor.matmul(
                        sc[:, j * NC_CHUNK : (j + 1) * NC_CHUNK],
                        lhsT=kt_h[:, tb * 128 : (tb + 1) * 128],
                        rhs=qt_h[
                            :, s0 + j * NC_CHUNK : s0 + (j + 1) * NC_CHUNK
                        ],
                        start=True,
                        stop=True,
                    )
                nc.scalar.activation(
                    attnT[:, tb, :], sc, mybir.ActivationFunctionType.Exp, scale=SCALE
                )
                for c in range(SW // NC_CHUNK):
                    nc.tensor.matmul(
                        zts[c],
                        lhsT=v_h[:, tb, :],
                        rhs=attnT[:, tb, c * NC_CHUNK : (c + 1) * NC_CHUNK],
                        start=(tb == 0),
                        stop=(tb == TB - 1),
                    )
                if tb >= 8:
                    i = tb - 8
                    nc.vector.tensor_add(
                        f1[:, i, :], attnT[:, i, :], attnT[:, tb, :]
                    )
                if tb >= 12:
                    i = tb - 12
                    nc.vector.tensor_add(f2[:, i, :], f1[:, i, :], f1[:, i + 4, :])

            for c in range(SW // NC_CHUNK):
                c0 = c * NC_CHUNK
                # ---- denominator (column sums of attnT) ----
                dn = psum_work.tile([128, NC_CHUNK], F32, tag="work")
                for j in range(TB // 4):
                    nc.tensor.matmul(
                        dn,
                        lhsT=ones_bf,
                        rhs=f2[:, j, c0 : c0 + NC_CHUNK],
                        start=(j == 0),
                        stop=(j == TB // 4 - 1),
                    )
                recip = works.tile([128, NC_CHUNK], F32, tag="recip")
                nc.vector.reciprocal_approx_fast(recip, dn)
                ztn = works.tile([128, NC_CHUNK], F16, tag="ztn")
                nc.vector.tensor_mul(ztn, zts[c], recip)
                # ---- output projection ----
                wo_ps = psum_work.tile([128, NC_CHUNK], F32, tag="work")
                nc.tensor.matmul(
                    wo_ps,
                    lhsT=wo_sb[:, h, :],
                    rhs=ztn,
                    start=True,
                    stop=True,
                )
                csl = slice(s0 + c0, s0 + c0 + NC_CHUNK)
                asl = slice(c0, c0 + NC_CHUNK)
                if h == 0:
                    nc.vector.tensor_copy(acc_a[sh][:, asl], wo_ps)
                elif h == 1:
                    nc.vector.tensor_add(acc_b[sh][:, asl], acc_a[sh][:, asl], wo_ps)
                elif h == 2:
                    nc.vector.tensor_add(acc_a[sh][:, asl], acc_b[sh][:, asl], wo_ps)
                else:
                    osb = works.tile([128, NC_CHUNK], F32, tag="osb")
                    nc.vector.tensor_add(osb, acc_a[sh][:, asl], wo_ps)
                    nc.sync.dma_start(out=out_d[:, csl], in_=osb)

    ctx.close()


def _in_maps(inputs):
    q = np.asarray(inputs["q"], dtype=np.float32)
    Wq = np.asarray(inputs["Wq"], dtype=np.float32)
    bq = np.asarray(inputs["bq"], dtype=np.float32)
    Wk = np.asarray(inputs["Wk"], dtype=np.float32)
    bk = np.asarray(inputs["bk"], dtype=np.float32)
    Wv = np.asarray(inputs["Wv"], dtype=np.float32)
    bv = np.asarray(inputs["bv"], dtype=np.float32)
    Wo = np.asarray(inputs["Wo"], dtype=np.float32).reshape(H, E, E)
    maps = []
    for c in range(8):
        b = c // 2
        hs = slice(4 * (c % 2), 4 * (c % 2) + 4)
        maps.append(
            {
                "q": np.ascontiguousarray(q[b]),
                "Wq": np.ascontiguousarray(Wq[hs]),
                "Wk": np.ascontiguousarray(Wk[hs]),
                "Wv": np.ascontiguousarray(Wv[hs]),
                "Wo": np.ascontiguousarray(Wo[hs]),
                "bq": np.ascontiguousarray(bq[hs]),
                "bk": np.ascontiguousarray(bk[hs]),
            }
        )
    return maps


def kernel(**inputs):
    nc = build_program()
    maps = _in_maps(inputs)
    res = run_bass_kernel_spmd(nc, maps, core_ids=list(range(8)))
    bo = np.asarray(inputs["bo"], dtype=np.float32)
    bv = np.asarray(inputs["bv"], dtype=np.float32)
    Wo = np.asarray(inputs["Wo"], dtype=np.float32).reshape(H, E, E)
    # V-bias contribution folded out of the device kernel:
    # sum_h softmax(..)@ (qWv + bv) @ Wo_h = device_partials + sum_h bv_h @ Wo_h
    bo_eff = bo + np.einsum("he,hef->f", bv, Wo).astype(np.float32)
    out = np.empty((B, S, E), dtype=np.float32)
    for b in range(B):
        part = res.results[2 * b]["out"] + res.results[2 * b + 1]["out"]
        out[b] = part.T + bo_eff
    return out
